# revision 10
# baseline (speedup 1.0000x reference)
"""Trainium2 Bass kernel for nn_ExpandEvecs.

Computes, for evecs [B=4, C=1, M=1024, K=32] and max_lvl=16, the stack of
cumulative low-rank reconstructions
    out[b, l] = V[:, :l+1] @ V[:, :l+1]^T      (V = evecs[b, 0, :, :max_lvl])
returned as [B, max_lvl, M, M] float32 (256 MiB full output).

Every level's matrix is SYMMETRIC, so the device only computes/writes the
upper-triangle 128-row blocks: row-block q (rows 128q..128q+127) covers
columns 128q..1023.  That is 36 of 64 blocks (56.25% of the bytes and of
the PE column streaming); assemble() mirrors the lower triangle on the
host (a numpy transpose-copy) and upcasts fp16 -> fp32.

SPMD trick: run_bass_kernel_spmd runs ONE program on all cores, so the
triangle is chopped into <=512-col chunks whose width multiset
{512x6, 384x2, 256x2, 128x2} splits into two IDENTICAL halves
{512x3, 384, 256, 128} = 2304 cols/level/core.  The host packs per-core
lhs/rhs input tensors (slices of vt) so the same static
6-chunks-per-level program computes either half; PIECES records the
chunk -> (row-block, col-range) map for host-side assembly.

PE row tiling: every matmul has contraction r = l+1 <= 16 <= 32, so four
matmuls placed at tile_position (32g, 0) for g = 0..3 run CONCURRENTLY
in the 128x128 array (measured ~3x for 4-tile K=32 packs).  The packed
lhs/rhs inputs are replicated by the host at partition offsets
0/32/64/96; chunk i uses row-group GROUPS[i].

Sharding: core c handles batch b = c//2, triangle half c % 2.

Inherited measured facts (full-matrix baseline + this kernel's traces):
- fp16 output passes the 2e-2 gate with ~3e-4 norm error.
- All output DMAs ride ONE HWDGE ring (sync); inputs ride the scalar
  ring.  Dual-ring output provokes a ring-host (DMA engine 15)
  head-of-line collision, bimodal +20-80%.
- Every output dma_start covers all 128 partitions (descriptor count 128
  -> HWDGE stripes packets over all 16 engines evenly).  Per-core DMA
  data bandwidth is ~343-360 GB/s; output is 9 MiB/core -> ~27 us floor,
  which is the roofline this kernel sits on.
- PSUM -> SBUF copies (with the fp32->fp16 cast) are the only engines
  that can drain PSUM (GPSIMD has no PSUM port): VectorE 1.04 ns/elem,
  ScalarE 0.83 ns/elem, ~125-145 ns fixed per instruction.  Copies are
  merged to 3 per level (1024 on V, 896 + 384 on S) via multi-bank PSUM
  tiles: pools 3x2-bank + 2x1-bank = exactly 8 banks.
"""

import sys

for _p in ("/root/.axon_site/_ro/trn_rl_repo", "/opt/trn_rl_repo"):
    if _p not in sys.path:
        sys.path.insert(0, _p)

import numpy as np

import concourse.bacc as bacc
import concourse.mybir as mybir
from concourse.tile import TileContext
from concourse import bass_utils

B, C, M, K, L = 4, 1, 1024, 32, 16
P = 128
F32 = mybir.dt.float32
F16 = mybir.dt.float16

# Static per-core chunk widths (identical on every core), and per-half
# (row-block q, col-offset-within-block cc) for each chunk.  Chunk i of
# half h computes out rows [128q, 128q+128) x cols [128q+cc, 128q+cc+w).
CHUNK_W = (512, 512, 512, 384, 128, 256)
W = sum(CHUNK_W)  # 2304 columns per level per core
PIECES = {
    0: ((0, 0), (1, 0), (2, 0), (5, 0), (7, 0), (6, 0)),
    1: ((0, 512), (3, 0), (4, 0), (1, 512), (3, 512), (2, 512)),
}
NCH = len(CHUNK_W)
GROUPS = (0, 1, 2, 3, 3, 0)  # PE row-group per chunk; chunks 3,4 share a
# PSUM bank and therefore a row-group (same-group matmuls serialize -- two
# CONCURRENT matmuls must never write the same PSUM bank)

OUT_BUFS = 4
DMA_MODE = "singles"  # "singles" | "pairs"
TILE4 = True


def build_nc(out_bufs=None, dma_mode=None, tile4=None):
    if out_bufs is None:
        out_bufs = OUT_BUFS
    if dma_mode is None:
        dma_mode = DMA_MODE
    if tile4 is None:
        tile4 = TILE4
    nc = bacc.Bacc("TRN2", target_bir_lowering=False, debug=False)
    lhs_d = nc.dram_tensor("lhs", [4, L, NCH * P], F16, kind="ExternalInput")
    rhs_d = nc.dram_tensor("rhs", [4, L, W], F16, kind="ExternalInput")
    out = nc.dram_tensor("out", [P, L * W], F16, kind="ExternalOutput")
    out_v = out.ap().rearrange("p (l w) -> p l w", w=W)

    # static chunk offsets within a level's packed output row
    offs = [0]
    for w in CHUNK_W:
        offs.append(offs[-1] + w)

    with TileContext(nc) as tc:
        with (
            tc.tile_pool(name="consts", bufs=1) as consts,
            tc.tile_pool(name="outp", bufs=out_bufs) as outp,
            tc.tile_pool(name="psum", bufs=4, space="PSUM") as psump,
        ):
            # Packed inputs, replicated by the host at partition offsets
            # 0/32/64/96 (PE row-groups).  Ride the scalar ring, split so
            # level-0 compute starts as soon as the first rows land.
            lhs = consts.tile([P, NCH * P], F16)
            rhs = consts.tile([P, W], F16)
            # prefix rows 0:4 first so levels 0-3 start as soon as ~100KB
            # lands; the rest follows.
            # Input dma_starts ride a gpsimd-hosted queue: issuing a dma
            # costs the hosting engine time, and scalar/vector must start
            # level-0 copies ASAP while the sync ring is FIFO (any input
            # dma_start queued there delays every output level behind it).
            for g in range(4):
                nc.gpsimd.dma_start(out=lhs[32 * g : 32 * g + L, :], in_=lhs_d.ap()[g])
                nc.gpsimd.dma_start(out=rhs[32 * g : 32 * g + L, :], in_=rhs_d.ap()[g])

            ot = None
            for l in range(L):
                r = l + 1
                if dma_mode == "pairs":
                    s = l % 2
                    if s == 0:
                        ot = outp.tile([P, 2 * W], F16)
                else:
                    s = 0
                    ot = outp.tile([P, W], F16)
                so = s * W
                # three PSUM tiles per level; chunk j of the level lands at
                # [a1 | a1 | a2 | a2 | b | b] so copies merge to 3.
                a1 = psump.tile([P, 1024], F32, name="ps")
                a2 = psump.tile([P, 1024], F32, name="ps")
                bb = psump.tile([P, 1024], F32, name="ps")
                dsts = (
                    a1[:, 0:512],
                    a1[:, 512:1024],
                    a2[:, 0:512],
                    a2[:, 512:896],
                    a2[:, 896:1024],
                    bb[:, 0:256],
                )
                for i, wch in enumerate(CHUNK_W):
                    g = GROUPS[i] if tile4 else 0
                    p0 = 32 * g
                    nc.tensor.matmul(
                        dsts[i],
                        lhs[p0 : p0 + r, i * P : (i + 1) * P],
                        rhs[p0 : p0 + r, offs[i] : offs[i] + wch],
                        start=True,
                        stop=True,
                        tile_position=(p0, 0) if tile4 else None,
                    )
                nc.vector.tensor_copy(out=ot[:, so : so + 1024], in_=a1[:, :])
                nc.scalar.copy(out=ot[:, so + 1024 : so + 2048], in_=a2[:, :])
                nc.scalar.copy(out=ot[:, so + 2048 : so + 2304], in_=bb[:, 0:256])
                if dma_mode == "pairs":
                    if l in (0, 1):
                        nc.sync.dma_start(
                            out=out_v[:, l : l + 1, :],
                            in_=ot[:, so : so + W].rearrange("p (l w) -> p l w", l=1),
                        )
                    elif s == 1:
                        nc.sync.dma_start(
                            out=out_v[:, l - 1 : l + 1, :],
                            in_=ot[:, :].rearrange("p (l w) -> p l w", w=W),
                        )
                else:
                    nc.sync.dma_start(
                        out=out_v[:, l : l + 1, :],
                        in_=ot[:, :].rearrange("p (l w) -> p l w", l=1),
                    )
    nc.compile()
    return nc


_NC_CACHE = {}


def _get_nc():
    key = (OUT_BUFS, DMA_MODE, TILE4)
    if key not in _NC_CACHE:
        _NC_CACHE[key] = build_nc()
    return _NC_CACHE[key]


def make_in_maps(evecs):
    evecs = np.asarray(evecs, dtype=np.float32)
    in_maps = []
    for core in range(8):
        b, half = core // 2, core % 2
        vt = np.ascontiguousarray(evecs[b, 0, :, :L].T).astype(np.float16)
        lhs = np.empty((L, NCH * P), dtype=np.float16)
        rhs = np.empty((L, W), dtype=np.float16)
        off = 0
        for i, (q, cc) in enumerate(PIECES[half]):
            w = CHUNK_W[i]
            lhs[:, i * P : (i + 1) * P] = vt[:, 128 * q : 128 * (q + 1)]
            rhs[:, off : off + w] = vt[:, 128 * q + cc : 128 * q + cc + w]
            off += w
        in_maps.append(
            {
                "lhs": np.ascontiguousarray(np.broadcast_to(lhs, (4, L, NCH * P))),
                "rhs": np.ascontiguousarray(np.broadcast_to(rhs, (4, L, W))),
            }
        )
    return in_maps


def assemble(results):
    fullh = np.empty((B, L * C, M, M), dtype=np.float16)
    for core in range(8):
        b, half = core // 2, core % 2
        arr = results[core]["out"].reshape(P, L, W)
        off = 0
        for i, (q, cc) in enumerate(PIECES[half]):
            w = CHUNK_W[i]
            c0 = 128 * q + cc
            fullh[b, :, 128 * q : 128 * (q + 1), c0 : c0 + w] = arr[
                :, :, off : off + w
            ].transpose(1, 0, 2)
            off += w
    # mirror the lower triangle (every level's matrix is symmetric)
    for i in range(8):
        si = slice(128 * i, 128 * (i + 1))
        for j in range(i + 1, 8):
            sj = slice(128 * j, 128 * (j + 1))
            fullh[:, :, sj, si] = fullh[:, :, si, sj].swapaxes(-1, -2)
    return fullh.astype(np.float32)


def kernel(evecs, max_lvl):
    assert int(max_lvl) == L, f"kernel hardcodes max_lvl={L}, got {max_lvl}"
    nc = _get_nc()
    res = bass_utils.run_bass_kernel_spmd(nc, make_in_maps(evecs), list(range(8)))
    return assemble(res.results)


# revision 11
# speedup vs baseline: 1.0799x; 1.0799x over previous
"""Trainium2 Bass kernel for nn_ExpandEvecs.

Computes, for evecs [B=4, C=1, M=1024, K=32] and max_lvl=16, the stack of
cumulative low-rank reconstructions
    out[b, l] = V[:, :l+1] @ V[:, :l+1]^T      (V = evecs[b, 0, :, :max_lvl])
returned as [B, max_lvl, M, M] float32 (256 MiB full output).

Every level's matrix is SYMMETRIC, so the device only computes/writes the
upper-triangle 128-row blocks: row-block q (rows 128q..128q+127) covers
columns 128q..1023.  That is 36 of 64 blocks (56.25% of the bytes and of
the PE column streaming); assemble() mirrors the lower triangle on the
host (a numpy transpose-copy) and upcasts fp16 -> fp32.

SPMD trick: run_bass_kernel_spmd runs ONE program on all cores, so the
triangle is chopped into <=512-col chunks whose width multiset
{512x6, 384x2, 256x2, 128x2} splits into two IDENTICAL halves
{512x3, 384, 256, 128} = 2304 cols/level/core.  The host packs per-core
lhs/rhs input tensors (slices of vt) so the same static
6-chunks-per-level program computes either half; PIECES records the
chunk -> (row-block, col-range) map for host-side assembly.

PE row tiling: every matmul has contraction r = l+1 <= 16 <= 32, so four
matmuls placed at tile_position (32g, 0) for g = 0..3 run CONCURRENTLY
in the 128x128 array (measured ~3x for 4-tile K=32 packs).  The packed
lhs/rhs inputs are replicated by the host at partition offsets
0/32/64/96; chunk i uses row-group GROUPS[i].

Sharding: core c handles batch b = c//2, triangle half c % 2.

Inherited measured facts (full-matrix baseline + this kernel's traces):
- fp16 output passes the 2e-2 gate with ~3e-4 norm error.
- All output DMAs ride ONE HWDGE ring (sync); inputs ride the scalar
  ring.  Dual-ring output provokes a ring-host (DMA engine 15)
  head-of-line collision, bimodal +20-80%.
- Every output dma_start covers all 128 partitions (descriptor count 128
  -> HWDGE stripes packets over all 16 engines evenly).  Per-core DMA
  data bandwidth is ~343-360 GB/s; output is 9 MiB/core -> ~27 us floor,
  which is the roofline this kernel sits on.
- PSUM -> SBUF copies (with the fp32->fp16 cast) are the only engines
  that can drain PSUM (GPSIMD has no PSUM port): VectorE 1.04 ns/elem,
  ScalarE 0.83 ns/elem, ~125-145 ns fixed per instruction.  Copies are
  merged to 3 per level (1024 on V, 896 + 384 on S) via multi-bank PSUM
  tiles: pools 3x2-bank + 2x1-bank = exactly 8 banks.
"""

import sys

for _p in ("/root/.axon_site/_ro/trn_rl_repo", "/opt/trn_rl_repo"):
    if _p not in sys.path:
        sys.path.insert(0, _p)

import numpy as np

import concourse.bacc as bacc
import concourse.mybir as mybir
from concourse.tile import TileContext
from concourse import bass_utils

B, C, M, K, L = 4, 1, 1024, 32, 16
P = 128
F32 = mybir.dt.float32
F16 = mybir.dt.float16

# Static per-core chunk widths (identical on every core), and per-half
# (row-block q, col-offset-within-block cc) for each chunk.  Chunk i of
# half h computes out rows [128q, 128q+128) x cols [128q+cc, 128q+cc+w).
CHUNK_W = (512, 512, 512, 384, 128, 256)
W = sum(CHUNK_W)  # 2304 columns per level per core
PIECES = {
    0: ((0, 0), (1, 0), (2, 0), (5, 0), (7, 0), (6, 0)),
    1: ((0, 512), (3, 0), (4, 0), (1, 512), (3, 512), (2, 512)),
}
NCH = len(CHUNK_W)
GROUPS = (0, 1, 2, 3, 3, 0)  # PE row-group per chunk; chunks 3,4 share a
# PSUM bank and therefore a row-group (same-group matmuls serialize -- two
# CONCURRENT matmuls must never write the same PSUM bank)

OUT_BUFS = 4
DMA_MODE = "singles"  # "singles" | "pairs"
TILE4 = True


def build_nc(out_bufs=None, dma_mode=None, tile4=None):
    if out_bufs is None:
        out_bufs = OUT_BUFS
    if dma_mode is None:
        dma_mode = DMA_MODE
    if tile4 is None:
        tile4 = TILE4
    nc = bacc.Bacc("TRN2", target_bir_lowering=False, debug=False)
    # pre: rows 0:4 of [lhs | rhs] (levels 0-1, untiled, lands instantly);
    # rep: full [lhs | rhs] replicated at partition offsets 0/32/64/96.
    C = NCH * P + W
    pre_d = nc.dram_tensor("pre", [4, C], F16, kind="ExternalInput")
    rep_d = nc.dram_tensor("rep", [P, C], F16, kind="ExternalInput")
    out = nc.dram_tensor("out", [P, L * W], F16, kind="ExternalOutput")
    out_v = out.ap().rearrange("p (l w) -> p l w", w=W)

    # static chunk offsets within a level's packed output row
    offs = [0]
    for w in CHUNK_W:
        offs.append(offs[-1] + w)

    with TileContext(nc) as tc:
        with (
            tc.tile_pool(name="consts", bufs=1) as consts,
            tc.tile_pool(name="outp", bufs=out_bufs) as outp,
            tc.tile_pool(name="psum", bufs=4, space="PSUM") as psump,
        ):
            # Packed inputs, replicated by the host at partition offsets
            # 0/32/64/96 (PE row-groups).  Ride the scalar ring, split so
            # level-0 compute starts as soon as the first rows land.
            # prefix rows 0:4 first so levels 0-3 start as soon as ~100KB
            # lands; the rest follows.
            # Exactly TWO input dma_starts: each dma_start costs its hosting
            # engine ~600ns to issue and the rings are FIFO, so input count
            # directly delays either the copies (scalar host) or the output
            # stream (sync host).  The tiny pre tile unblocks levels 0-1.
            pre = consts.tile([4, C], F16)
            rep = consts.tile([P, C], F16)
            nc.scalar.dma_start(out=pre[0:4, :], in_=pre_d.ap()[0:4])
            nc.sync.dma_start(out=rep[0:P, :], in_=rep_d.ap()[0:P])

            ot = None
            for l in range(L):
                r = l + 1
                if dma_mode == "pairs":
                    s = l % 2
                    if s == 0:
                        ot = outp.tile([P, 2 * W], F16)
                else:
                    s = 0
                    ot = outp.tile([P, W], F16)
                so = s * W
                # three PSUM tiles per level; chunk j of the level lands at
                # [a1 | a1 | a2 | a2 | b | b] so copies merge to 3.
                a1 = psump.tile([P, 1024], F32, name="ps")
                a2 = psump.tile([P, 1024], F32, name="ps")
                bb = psump.tile([P, 1024], F32, name="ps")
                dsts = (
                    a1[:, 0:512],
                    a1[:, 512:1024],
                    a2[:, 0:512],
                    a2[:, 512:896],
                    a2[:, 896:1024],
                    bb[:, 0:256],
                )
                for i, wch in enumerate(CHUNK_W):
                    early = l < 2
                    g = GROUPS[i] if (tile4 and not early) else 0
                    p0 = 32 * g
                    src_t = pre if early else rep
                    nc.tensor.matmul(
                        dsts[i],
                        src_t[p0 : p0 + r, i * P : (i + 1) * P],
                        src_t[p0 : p0 + r, NCH * P + offs[i] : NCH * P + offs[i] + wch],
                        start=True,
                        stop=True,
                        tile_position=(p0, 0) if (tile4 and not early) else None,
                    )
                nc.vector.tensor_copy(out=ot[:, so : so + 1024], in_=a1[:, :])
                nc.scalar.copy(out=ot[:, so + 1024 : so + 2048], in_=a2[:, :])
                nc.scalar.copy(out=ot[:, so + 2048 : so + 2304], in_=bb[:, 0:256])
                if dma_mode == "pairs":
                    if l in (0, 1):
                        nc.sync.dma_start(
                            out=out_v[:, l : l + 1, :],
                            in_=ot[:, so : so + W].rearrange("p (l w) -> p l w", l=1),
                        )
                    elif s == 1:
                        nc.sync.dma_start(
                            out=out_v[:, l - 1 : l + 1, :],
                            in_=ot[:, :].rearrange("p (l w) -> p l w", w=W),
                        )
                elif l < 2:
                    # ramp: stream the V-copied half as soon as it lands
                    ovl = out_v[:, l, :].rearrange("p (x w) -> p x w", x=1)
                    nc.sync.dma_start(
                        out=ovl[:, :, 0:1024],
                        in_=ot[:, 0:1024].rearrange("p (x w) -> p x w", x=1),
                    )
                    nc.sync.dma_start(
                        out=ovl[:, :, 1024:W],
                        in_=ot[:, 1024:W].rearrange("p (x w) -> p x w", x=1),
                    )
                else:
                    nc.sync.dma_start(
                        out=out_v[:, l : l + 1, :],
                        in_=ot[:, :].rearrange("p (l w) -> p l w", l=1),
                    )
    nc.compile()
    return nc


_NC_CACHE = {}


def _get_nc():
    key = (OUT_BUFS, DMA_MODE, TILE4)
    if key not in _NC_CACHE:
        _NC_CACHE[key] = build_nc()
    return _NC_CACHE[key]


def make_in_maps(evecs):
    evecs = np.asarray(evecs, dtype=np.float32)
    in_maps = []
    for core in range(8):
        b, half = core // 2, core % 2
        vt = np.ascontiguousarray(evecs[b, 0, :, :L].T).astype(np.float16)
        lhs = np.empty((L, NCH * P), dtype=np.float16)
        rhs = np.empty((L, W), dtype=np.float16)
        off = 0
        for i, (q, cc) in enumerate(PIECES[half]):
            w = CHUNK_W[i]
            lhs[:, i * P : (i + 1) * P] = vt[:, 128 * q : 128 * (q + 1)]
            rhs[:, off : off + w] = vt[:, 128 * q + cc : 128 * q + cc + w]
            off += w
        cat = np.concatenate([lhs, rhs], axis=1)  # [L, C]
        rep = np.zeros((P, cat.shape[1]), dtype=np.float16)
        for g in range(4):
            rep[32 * g : 32 * g + L] = cat
        in_maps.append({"pre": np.ascontiguousarray(cat[0:4]), "rep": rep})
    return in_maps


def assemble(results):
    fullh = np.empty((B, L * C, M, M), dtype=np.float16)
    for core in range(8):
        b, half = core // 2, core % 2
        arr = results[core]["out"].reshape(P, L, W)
        off = 0
        for i, (q, cc) in enumerate(PIECES[half]):
            w = CHUNK_W[i]
            c0 = 128 * q + cc
            fullh[b, :, 128 * q : 128 * (q + 1), c0 : c0 + w] = arr[
                :, :, off : off + w
            ].transpose(1, 0, 2)
            off += w
    # mirror the lower triangle (every level's matrix is symmetric)
    for i in range(8):
        si = slice(128 * i, 128 * (i + 1))
        for j in range(i + 1, 8):
            sj = slice(128 * j, 128 * (j + 1))
            fullh[:, :, sj, si] = fullh[:, :, si, sj].swapaxes(-1, -2)
    return fullh.astype(np.float32)


def kernel(evecs, max_lvl):
    assert int(max_lvl) == L, f"kernel hardcodes max_lvl={L}, got {max_lvl}"
    nc = _get_nc()
    res = bass_utils.run_bass_kernel_spmd(nc, make_in_maps(evecs), list(range(8)))
    return assemble(res.results)


# revision 12
# speedup vs baseline: 1.1606x; 1.0748x over previous
"""Trainium2 Bass kernel for nn_ExpandEvecs.

Computes, for evecs [B=4, C=1, M=1024, K=32] and max_lvl=16, the stack of
cumulative low-rank reconstructions
    out[b, l] = V[:, :l+1] @ V[:, :l+1]^T      (V = evecs[b, 0, :, :max_lvl])
returned as [B, max_lvl, M, M] float32 (256 MiB full output).

Every level's matrix is SYMMETRIC, so the device only computes/writes the
upper-triangle 128-row blocks: row-block q (rows 128q..128q+127) covers
columns 128q..1023.  That is 36 of 64 blocks (56.25% of the bytes and of
the PE column streaming); assemble() mirrors the lower triangle on the
host (a numpy transpose-copy) and upcasts fp16 -> fp32.

SPMD trick: run_bass_kernel_spmd runs ONE program on all cores, so the
triangle is chopped into <=512-col chunks whose width multiset
{512x6, 384x2, 256x2, 128x2} splits into two IDENTICAL halves
{512x3, 384, 256, 128} = 2304 cols/level/core.  The host packs per-core
lhs/rhs input tensors (slices of vt) so the same static
6-chunks-per-level program computes either half; PIECES records the
chunk -> (row-block, col-range) map for host-side assembly.

PE row tiling: every matmul has contraction r = l+1 <= 16 <= 32, so four
matmuls placed at tile_position (32g, 0) for g = 0..3 run CONCURRENTLY
in the 128x128 array (measured ~3x for 4-tile K=32 packs).  The packed
lhs/rhs inputs are replicated by the host at partition offsets
0/32/64/96; chunk i uses row-group GROUPS[i].

Sharding: core c handles batch b = c//2, triangle half c % 2.

Inherited measured facts (full-matrix baseline + this kernel's traces):
- fp16 output passes the 2e-2 gate with ~3e-4 norm error.
- All output DMAs ride ONE HWDGE ring (sync); inputs ride the scalar
  ring.  Dual-ring output provokes a ring-host (DMA engine 15)
  head-of-line collision, bimodal +20-80%.
- Every output dma_start covers all 128 partitions (descriptor count 128
  -> HWDGE stripes packets over all 16 engines evenly).  Per-core DMA
  data bandwidth is ~343-360 GB/s; output is 9 MiB/core -> ~27 us floor,
  which is the roofline this kernel sits on.
- PSUM -> SBUF copies (with the fp32->fp16 cast) are the only engines
  that can drain PSUM (GPSIMD has no PSUM port): VectorE 1.04 ns/elem,
  ScalarE 0.83 ns/elem, ~125-145 ns fixed per instruction.  Copies are
  merged to 3 per level (1024 on V, 896 + 384 on S) via multi-bank PSUM
  tiles: pools 3x2-bank + 2x1-bank = exactly 8 banks.
"""

import sys

for _p in ("/root/.axon_site/_ro/trn_rl_repo", "/opt/trn_rl_repo"):
    if _p not in sys.path:
        sys.path.insert(0, _p)

import numpy as np

import concourse.bacc as bacc
import concourse.mybir as mybir
from concourse.tile import TileContext
from concourse import bass_utils

B, C, M, K, L = 4, 1, 1024, 32, 16
P = 128
F32 = mybir.dt.float32
F16 = mybir.dt.float16

# Static per-core chunk widths (identical on every core), and per-half
# (row-block q, col-offset-within-block cc) for each chunk.  Chunk i of
# half h computes out rows [128q, 128q+128) x cols [128q+cc, 128q+cc+w).
CHUNK_W = (512, 512, 512, 384, 128, 256)
W = sum(CHUNK_W)  # 2304 columns per level per core
PIECES = {
    0: ((0, 0), (1, 0), (2, 0), (5, 0), (7, 0), (6, 0)),
    1: ((0, 512), (3, 0), (4, 0), (1, 512), (3, 512), (2, 512)),
}
NCH = len(CHUNK_W)
GROUPS = (0, 1, 2, 3, 3, 0)  # PE row-group per chunk; chunks 3,4 share a
# PSUM bank and therefore a row-group (same-group matmuls serialize -- two
# CONCURRENT matmuls must never write the same PSUM bank)

OUT_BUFS = 4
DMA_MODE = "singles"  # "singles" | "pairs"
TILE4 = True


def build_nc(out_bufs=None, dma_mode=None, tile4=None):
    if out_bufs is None:
        out_bufs = OUT_BUFS
    if dma_mode is None:
        dma_mode = DMA_MODE
    if tile4 is None:
        tile4 = TILE4
    nc = bacc.Bacc("TRN2", target_bir_lowering=False, debug=False)
    # pre: rows 0:4 of [lhs | rhs] (levels 0-1, untiled, lands instantly);
    # rep: full [lhs | rhs] replicated at partition offsets 0/32/64/96.
    C = NCH * P + W
    pre_d = nc.dram_tensor("pre", [4, C], F16, kind="ExternalInput")
    rep_d = nc.dram_tensor("rep", [P, C], F16, kind="ExternalInput")
    out = nc.dram_tensor("out", [P, L * W], F16, kind="ExternalOutput")
    out_v = out.ap().rearrange("p (l w) -> p l w", w=W)

    # static chunk offsets within a level's packed output row
    offs = [0]
    for w in CHUNK_W:
        offs.append(offs[-1] + w)

    with TileContext(nc) as tc:
        with (
            tc.tile_pool(name="consts", bufs=1) as consts,
            tc.tile_pool(name="outp", bufs=out_bufs) as outp,
            tc.tile_pool(name="psum", bufs=4, space="PSUM") as psump,
        ):
            # Packed inputs, replicated by the host at partition offsets
            # 0/32/64/96 (PE row-groups).  Ride the scalar ring, split so
            # level-0 compute starts as soon as the first rows land.
            # prefix rows 0:4 first so levels 0-3 start as soon as ~100KB
            # lands; the rest follows.
            # Exactly TWO input dma_starts: each dma_start costs its hosting
            # engine ~600ns to issue and the rings are FIFO, so input count
            # directly delays either the copies (scalar host) or the output
            # stream (sync host).  The tiny pre tile unblocks levels 0-1.
            pre = consts.tile([4, C], F16)
            rep = consts.tile([P, C], F16)
            nc.sync.dma_start(out=pre[0:4, :], in_=pre_d.ap()[0:4])
            nc.sync.dma_start(out=rep[0:P, :], in_=rep_d.ap()[0:P])

            ot = None
            for l in range(L):
                r = l + 1
                if dma_mode == "pairs":
                    s = l % 2
                    if s == 0:
                        ot = outp.tile([P, 2 * W], F16)
                else:
                    s = 0
                    ot = outp.tile([P, W], F16)
                so = s * W
                # three PSUM tiles per level; chunk j of the level lands at
                # [a1 | a1 | a2 | a2 | b | b] so copies merge to 3.
                a1 = psump.tile([P, 1024], F32, name="ps")
                a2 = psump.tile([P, 1024], F32, name="ps")
                bb = psump.tile([P, 1024], F32, name="ps")
                dsts = (
                    a1[:, 0:512],
                    a1[:, 512:1024],
                    a2[:, 0:512],
                    a2[:, 512:896],
                    a2[:, 896:1024],
                    bb[:, 0:256],
                )
                for i, wch in enumerate(CHUNK_W):
                    early = l < 2
                    g = GROUPS[i] if (tile4 and not early) else 0
                    p0 = 32 * g
                    src_t = pre if early else rep
                    nc.tensor.matmul(
                        dsts[i],
                        src_t[p0 : p0 + r, i * P : (i + 1) * P],
                        src_t[p0 : p0 + r, NCH * P + offs[i] : NCH * P + offs[i] + wch],
                        start=True,
                        stop=True,
                        tile_position=(p0, 0) if (tile4 and not early) else None,
                    )
                nc.vector.tensor_copy(out=ot[:, so : so + 1024], in_=a1[:, :])
                nc.scalar.copy(out=ot[:, so + 1024 : so + 2048], in_=a2[:, :])
                nc.scalar.copy(out=ot[:, so + 2048 : so + 2304], in_=bb[:, 0:256])
                if dma_mode == "pairs":
                    if l in (0, 1):
                        nc.sync.dma_start(
                            out=out_v[:, l : l + 1, :],
                            in_=ot[:, so : so + W].rearrange("p (l w) -> p l w", l=1),
                        )
                    elif s == 1:
                        nc.sync.dma_start(
                            out=out_v[:, l - 1 : l + 1, :],
                            in_=ot[:, :].rearrange("p (l w) -> p l w", w=W),
                        )
                elif l < 2:
                    # ramp: stream the V-copied half as soon as it lands
                    ovl = out_v[:, l, :].rearrange("p (x w) -> p x w", x=1)
                    nc.sync.dma_start(
                        out=ovl[:, :, 0:1024],
                        in_=ot[:, 0:1024].rearrange("p (x w) -> p x w", x=1),
                    )
                    nc.sync.dma_start(
                        out=ovl[:, :, 1024:W],
                        in_=ot[:, 1024:W].rearrange("p (x w) -> p x w", x=1),
                    )
                else:
                    nc.sync.dma_start(
                        out=out_v[:, l : l + 1, :],
                        in_=ot[:, :].rearrange("p (l w) -> p l w", l=1),
                    )
    nc.compile()
    return nc


_NC_CACHE = {}


def _get_nc():
    key = (OUT_BUFS, DMA_MODE, TILE4)
    if key not in _NC_CACHE:
        _NC_CACHE[key] = build_nc()
    return _NC_CACHE[key]


def make_in_maps(evecs):
    evecs = np.asarray(evecs, dtype=np.float32)
    in_maps = []
    for core in range(8):
        b, half = core // 2, core % 2
        vt = np.ascontiguousarray(evecs[b, 0, :, :L].T).astype(np.float16)
        lhs = np.empty((L, NCH * P), dtype=np.float16)
        rhs = np.empty((L, W), dtype=np.float16)
        off = 0
        for i, (q, cc) in enumerate(PIECES[half]):
            w = CHUNK_W[i]
            lhs[:, i * P : (i + 1) * P] = vt[:, 128 * q : 128 * (q + 1)]
            rhs[:, off : off + w] = vt[:, 128 * q + cc : 128 * q + cc + w]
            off += w
        cat = np.concatenate([lhs, rhs], axis=1)  # [L, C]
        rep = np.zeros((P, cat.shape[1]), dtype=np.float16)
        for g in range(4):
            rep[32 * g : 32 * g + L] = cat
        in_maps.append({"pre": np.ascontiguousarray(cat[0:4]), "rep": rep})
    return in_maps


def assemble(results):
    fullh = np.empty((B, L * C, M, M), dtype=np.float16)
    for core in range(8):
        b, half = core // 2, core % 2
        arr = results[core]["out"].reshape(P, L, W)
        off = 0
        for i, (q, cc) in enumerate(PIECES[half]):
            w = CHUNK_W[i]
            c0 = 128 * q + cc
            fullh[b, :, 128 * q : 128 * (q + 1), c0 : c0 + w] = arr[
                :, :, off : off + w
            ].transpose(1, 0, 2)
            off += w
    # mirror the lower triangle (every level's matrix is symmetric)
    for i in range(8):
        si = slice(128 * i, 128 * (i + 1))
        for j in range(i + 1, 8):
            sj = slice(128 * j, 128 * (j + 1))
            fullh[:, :, sj, si] = fullh[:, :, si, sj].swapaxes(-1, -2)
    return fullh.astype(np.float32)


def kernel(evecs, max_lvl):
    assert int(max_lvl) == L, f"kernel hardcodes max_lvl={L}, got {max_lvl}"
    nc = _get_nc()
    res = bass_utils.run_bass_kernel_spmd(nc, make_in_maps(evecs), list(range(8)))
    return assemble(res.results)
